# revision 4
# baseline (speedup 1.0000x reference)
"""Trainium2 Bass kernel for a batched 4-dim Kalman filter step.

The reference computes, for B = 4,194,304 independent state columns:
    g_update = g_predict + K (z - psi g_predict),  g_predict = phi g
    z_update = psi g_update
    H_update = H_predict - K psi H_predict          (tiny, batch-independent)

All batch-dim work is affine in [g; z]:  out6 = M6 @ in6 with a 6x6 M6
computed host-side from the tiny matrices (phi, psi, H, N).  The device
kernel streams in6 [6, B] -> out6 [6, B] through a block-diagonal
TensorE matmul: 16 column-groups are packed along 96 SBUF partitions
(partition p = row*16 + group), so one [96,96] x [96,512] fp32 matmul
processes 16*512 = 8192 batch columns.

Sharding: pure data parallel over 8 NeuronCores (batch split 8 ways,
524,288 columns per core); the tiny W96 operand is replicated.
"""

import os
import sys

import numpy as np

for _p in ("/opt/trn_rl_repo",):
    if _p not in sys.path:
        sys.path.insert(0, _p)

import concourse.bass as bass
import concourse.bacc as bacc
import concourse.mybir as mybir
from concourse.tile import TileContext
from concourse.bass_utils import run_bass_kernel_spmd

B = 4_194_304
NCORES = 8
B8 = B // NCORES            # 524,288 batch columns per core
G = 16                      # column-groups packed along partitions
RIN = 6                     # rows: 4 of g + 2 of z
KP = RIN * G                # 96 partitions used (of 128)
C = B8 // G                 # 32,768 free-dim columns of the packed [96, C] layout
F = 2048                    # free-dim tile width (per-tile DMA = 96*F*4 = 768 KB)
NMM = 512                   # fp32 moving-operand max / one PSUM bank

LAST_RESULTS = None         # BassKernelResults of the most recent run (for test.py)


def build_nc(c_cols=C, f=F, mm_dtype=mybir.dt.float32):
    """One-core Bass program (SPMD-replicated across the 8 cores)."""
    nc = bacc.Bacc(None, target_bir_lowering=False)
    dt = mybir.dt.float32
    in96 = nc.declare_dram_parameter("in96", [KP, c_cols], dt, isOutput=False)
    w96 = nc.declare_dram_parameter("w96", [KP, KP], dt, isOutput=False)
    out96 = nc.declare_dram_parameter("out96", [KP, c_cols], dt, isOutput=True)

    ntiles = c_cols // f
    nmm = f // NMM
    with TileContext(nc) as tc:
        with (
            tc.tile_pool(name="wp", bufs=1) as wp,
            tc.tile_pool(name="inp", bufs=4) as inp,
            tc.tile_pool(name="outp", bufs=4) as outp,
            tc.tile_pool(name="ps", bufs=8, space=bass.MemorySpace.PSUM) as ps,
        ):
            wt = wp.tile([KP, KP], mm_dtype)
            nc.sync.dma_start(out=wt[:], in_=w96[:, :])
            for t in range(ntiles):
                it = inp.tile([KP, f], mm_dtype)
                nc.sync.dma_start(out=it[:], in_=in96[:, t * f:(t + 1) * f])
                ot = outp.tile([KP, f], dt)
                for n in range(nmm):
                    pt = ps.tile([KP, NMM], dt)
                    nc.tensor.matmul(pt[:], wt[:], it[:, n * NMM:(n + 1) * NMM])
                    nc.vector.tensor_copy(ot[:, n * NMM:(n + 1) * NMM], pt[:])
                nc.scalar.dma_start(out=out96[:, t * f:(t + 1) * f], in_=ot[:])
    nc.compile()
    return nc


_NC_CACHE = {}


def _get_nc():
    key = (C, F)
    if key not in _NC_CACHE:
        _NC_CACHE[key] = build_nc()
    return _NC_CACHE[key]


def _tiny_mats(N, H, phi, psi):
    """Host-side f64 computation of M6 (the 6x6 affine map) and H_update."""
    phi64 = phi.astype(np.float64)
    psi64 = psi.astype(np.float64)
    H64 = H.astype(np.float64)
    N64 = N.astype(np.float64)
    Hp = phi64 @ H64 @ phi64.T
    S = psi64 @ Hp @ psi64.T + N64
    K = Hp @ psi64.T @ np.linalg.inv(S)
    A4 = (np.eye(4) - K @ psi64) @ phi64
    M6 = np.zeros((6, 6))
    M6[:4, :4] = A4
    M6[:4, 4:] = K
    M6[4:, :4] = psi64 @ A4
    M6[4:, 4:] = psi64 @ K
    Hu = (Hp - K @ (psi64 @ Hp)).astype(np.float32)
    return M6, Hu


def _pack_w96(M6):
    # lhsT[k=r*16+g, m=i*16+g] = M6[i, r]  (block-diagonal over the 16 groups)
    W96 = np.zeros((KP, KP), dtype=np.float32)
    idx = np.arange(G)
    for r in range(RIN):
        for i in range(RIN):
            W96[r * G + idx, i * G + idx] = np.float32(M6[i, r])
    return W96


def kernel(z, N, g, H, phi, psi):
    global LAST_RESULTS
    z = np.asarray(z, dtype=np.float32)
    g = np.asarray(g, dtype=np.float32)
    M6, H_update = _tiny_mats(np.asarray(N), np.asarray(H),
                              np.asarray(phi), np.asarray(psi))
    W96 = _pack_w96(M6)

    nc = _get_nc()
    in_maps = []
    for cid in range(NCORES):
        sl = slice(cid * B8, (cid + 1) * B8)
        packed = np.empty((KP, C), dtype=np.float32)
        # contiguous [4,B8] -> [64,C] and [2,B8] -> [32,C] reshapes are free
        packed[0:4 * G] = np.ascontiguousarray(g[:, sl]).reshape(4 * G, C)
        packed[4 * G:] = np.ascontiguousarray(z[:, sl]).reshape(2 * G, C)
        in_maps.append({"in96": packed, "w96": W96})

    trace = bool(int(os.environ.get("KALMAN_TRACE", "0")))
    res = run_bass_kernel_spmd(nc, in_maps, core_ids=list(range(NCORES)),
                               trace=trace)
    LAST_RESULTS = res

    g_update = np.empty((4, B), dtype=np.float32)
    z_update = np.empty((2, B), dtype=np.float32)
    for cid in range(NCORES):
        sl = slice(cid * B8, (cid + 1) * B8)
        out96 = res.results[cid]["out96"]
        g_update[:, sl] = out96[0:4 * G].reshape(4, B8)
        z_update[:, sl] = out96[4 * G:].reshape(2, B8)
    return (g_update, H_update, z_update)
